# revision 9
# baseline (speedup 1.0000x reference)
"""Sharded kNN retrieval kernel for Trainium2 (8 NeuronCores).

Strategy (v2, fp8 + grouped top-8):
  - Host: l2-normalize queries; queries bf16, img_memory fp8(e4m3, x16 scale);
    per-core transposed layout memt[c] = [128(d-part), 4(d-block), 25000(rows)].
  - Device (SPMD x8), per 2048-col block:
      PE:  sim = qT.T @ memT  (bf16 x fp8 -> PSUM f32)             ~2.1us
      DVE: 16:1 group max (tensor_reduce) straight off PSUM,        ~2.4us
           then MAX8 + FIND_INDEX8 over the 128 group maxes
      DMA: 1.05 MB fp8 per block                                    ~2.6us
    Stages pipeline; each core streams its 12.8 MB shard once.
  - Host: expand group candidates (x16 rows), rank by approx cos, exact f32
    rescore of the top rows; containment + duplicate-index checks with exact
    block-recompute fallback; assemble the reference output exactly.
"""

import numpy as np
import ml_dtypes

import concourse.bass as bass
import concourse.tile as tile
import concourse.mybir as mybir
from concourse import bass_utils

BF16 = ml_dtypes.bfloat16
FP8 = ml_dtypes.float8_e4m3

B = 128
D = 512
N = 200000
NCORES = 8
NSHARD = N // NCORES          # 25000
K = 3
ID_THRESHOLD = 0.15
SOFT_SCALE = 5.0
MSCALE = np.float32(16.0)     # img_memory scaled by 16 before fp8 quantization

WBLK = 2048
_full = NSHARD // WBLK        # 12
_rem = NSHARD - _full * WBLK  # 424
# (base, width, group_width)
BLOCKS = [(j * WBLK, WBLK, 16) for j in range(_full)] + \
         ([(NSHARD - _rem, _rem, 8)] if _rem else [])
NBLK = len(BLOCKS)            # 13

_NC_CACHE = {}


def _build_nc():
    if "nc" in _NC_CACHE:
        return _NC_CACHE["nc"]
    nc = bass.Bass("TRN2", target_bir_lowering=False, debug=False, num_devices=NCORES)
    qt = nc.dram_tensor("qt", [128, 4, 128], mybir.dt.bfloat16, kind="ExternalInput")
    # packed: block j occupies cols [4*base, 4*base+4*w) with b-major sublayout
    memt = nc.dram_tensor("memt", [128, 4 * NSHARD], mybir.dt.float8e4, kind="ExternalInput")
    vals_out = nc.dram_tensor("vals", [128, NBLK * 8], mybir.dt.float32, kind="ExternalOutput")
    idx_out = nc.dram_tensor("idx", [128, NBLK * 8], mybir.dt.uint16, kind="ExternalOutput")

    NSPLIT = 11 * 8   # result slots for blocks 0..10 (early output)

    with tile.TileContext(nc) as tc:
        with (
            tc.tile_pool(name="qt_pool", bufs=1) as qt_pool,
            tc.tile_pool(name="mem_s", bufs=2) as mem_s,
            tc.tile_pool(name="mem_a", bufs=2) as mem_a,
            tc.tile_pool(name="red_pool", bufs=2) as red_pool,
            tc.tile_pool(name="res_pool", bufs=1) as res_pool,
            tc.tile_pool(name="psum_pool", bufs=2, space="PSUM") as psum_pool,
        ):
            qt_tile = qt_pool.tile([128, 4, 128], mybir.dt.bfloat16)
            nc.scalar.dma_start(qt_tile[:], qt[:])

            vals_a = res_pool.tile([128, NSPLIT], mybir.dt.float32)
            idx_a = res_pool.tile([128, NSPLIT], mybir.dt.uint16)
            vals_b = res_pool.tile([128, NBLK * 8 - NSPLIT], mybir.dt.float32)
            idx_b = res_pool.tile([128, NBLK * 8 - NSPLIT], mybir.dt.uint16)

            # DMA schedule: blocks 0,1 split in halves across both queues for a
            # fast pipeline start; blocks 2..11 as two-block pair transfers
            # (16 KB/partition rows) alternating queues; remainder block last.
            tiles = {}          # block j -> (tile, byte offset of block in tile)

            def _dma(eng, pool, blks, tag):
                lo = 4 * BLOCKS[blks[0]][0]
                hi = 4 * (BLOCKS[blks[-1]][0] + BLOCKS[blks[-1]][1])
                t = pool.tile([128, hi - lo], mybir.dt.float8e4, tag=tag)
                eng.dma_start(t[:], memt[:, lo:hi])
                for bj in blks:
                    tiles[bj] = (t, 4 * BLOCKS[bj][0] - lo)

            # halves of blocks 0 and 1 (4 KB rows, arrive quickly)
            h01 = {}
            for bj in (0, 1):
                base = BLOCKS[bj][0]
                tl = mem_s.tile([128, 4096], mybir.dt.float8e4, tag="s")
                nc.sync.dma_start(tl[:], memt[:, 4 * base:4 * base + 4096])
                th = mem_a.tile([128, 4096], mybir.dt.float8e4, tag="a")
                nc.scalar.dma_start(th[:], memt[:, 4 * base + 4096:4 * base + 8192])
                h01[bj] = (tl, th)
            _dma(nc.sync, mem_s, [2, 3], "s")
            _dma(nc.scalar, mem_a, [4, 5], "a")
            _dma(nc.sync, mem_s, [6, 7], "s")
            _dma(nc.scalar, mem_a, [8, 9], "a")
            _dma(nc.sync, mem_s, [10, 11], "s")
            _dma(nc.scalar, mem_a, [12], "a")

            for j, (base, w, gw) in enumerate(BLOCKS):
                g = w // gw
                # chunk-major block layout: [chunk][b][sw] fp8, chunks of 512
                if j in h01:
                    tl, th = h01[j]
                    parts = [(tl, 0, 0), (tl, 512, 2048), (th, 1024, 0), (th, 1536, 2048)]
                else:
                    mt, boff = tiles[j]
                    parts = [(mt, s0, boff + (s0 // 512) * 2048) for s0 in range(0, w, 512)]
                ps = psum_pool.tile([128, g, gw], mybir.dt.float32, tag="ps")
                for mt, s0, off in parts:
                    sw = min(512, w - s0)
                    for b in range(4):
                        nc.tensor.matmul(
                            ps[:, s0 // gw:(s0 + sw) // gw, :],
                            qt_tile[:, b, :],
                            mt[:, off + b * sw:off + (b + 1) * sw],
                            start=(b == 0),
                            stop=(b == 3),
                        )
                red = red_pool.tile([128, g], mybir.dt.float32, tag="red")
                nc.vector.tensor_reduce(red[:], ps[:], axis=mybir.AxisListType.X,
                                        op=mybir.AluOpType.max)
                vt, it, col = (vals_a, idx_a, j * 8) if j * 8 < NSPLIT else \
                              (vals_b, idx_b, j * 8 - NSPLIT)
                nc.vector.max(vt[:, col:col + 8], red[:])
                nc.vector.max_index(it[:, col:col + 8], vt[:, col:col + 8], red[:])
                if j == 10:
                    nc.sync.dma_start(vals_out[:, 0:NSPLIT], vals_a[:])
                    nc.scalar.dma_start(idx_out[:, 0:NSPLIT], idx_a[:])

            nc.sync.dma_start(vals_out[:, NSPLIT:], vals_b[:])
            nc.scalar.dma_start(idx_out[:, NSPLIT:], idx_b[:])
    _split_excess_waits(nc)
    _NC_CACHE["nc"] = nc
    return nc


def _split_excess_waits(nc, keep=1):
    """Walrus's MM instruction struct fits only one embedded sync wait; move
    extra waits emitted by Tile onto standalone NoOps just before the MM."""
    ctr = 0
    for fn in nc.m.functions:
        for blk in fn.blocks:
            newl = []
            for inst in blk.instructions:
                si = inst.sync_info
                if (type(inst).__name__ != "InstNoOp" and si is not None
                        and si.on_wait and len(si.on_wait) > keep):
                    waits = list(si.on_wait)
                    for w in waits[:-keep]:
                        nop = mybir.InstNoOp(name=f"I-waitnop-{ctr}")
                        ctr += 1
                        nop.engine = inst.engine
                        nop.sync_info = mybir.SyncInfo(on_wait=[w], on_update=[])
                        newl.append(nop)
                    inst.sync_info = mybir.SyncInfo(
                        on_wait=waits[-keep:], on_update=list(si.on_update or []))
                newl.append(inst)
            blk.instructions = newl


def run_device_topk(qt_host, memt_cores, trace=False):
    """Run the SPMD device kernel.  Returns (vals [8,128,NBLK*8] f32 raw-dot,
    idx [8,128,NBLK*8] uint32 group indices, BassKernelResults)."""
    nc = _build_nc()
    in_maps = [{"qt": qt_host, "memt": memt_cores[c]} for c in range(NCORES)]
    res = bass_utils.run_bass_kernel_spmd(
        nc, in_maps, core_ids=list(range(NCORES)), trace=trace,
    )
    vals = np.stack([res.results[c]["vals"] for c in range(NCORES)]) / MSCALE
    idx = np.stack([res.results[c]["idx"] for c in range(NCORES)])
    return vals, idx, res


def _prep_inputs(i_feats, img_memory):
    qn = i_feats / np.linalg.norm(i_feats, axis=1, keepdims=True)
    qn = qn.astype(np.float32)
    qn_bf = qn.astype(BF16)
    qt_host = np.ascontiguousarray(qn_bf.reshape(B, 4, 128).transpose(2, 1, 0))

    def _pack(c):
        shard8 = (img_memory[c * NSHARD:(c + 1) * NSHARD] * MSCALE).astype(FP8)
        segs = []
        for base, w, _ in BLOCKS:
            for s0 in range(0, w, 512):         # chunk-major within each block
                sw = min(512, w - s0)
                seg = shard8[base + s0:base + s0 + sw].reshape(sw, 4, 128)
                segs.append(np.ascontiguousarray(seg.transpose(2, 1, 0)).reshape(128, 4 * sw))
        return np.concatenate(segs, axis=1)

    from concurrent.futures import ThreadPoolExecutor
    with ThreadPoolExecutor(max_workers=NCORES) as ex:
        memt_cores = list(ex.map(_pack, range(NCORES)))
    return qn, qn_bf, qt_host, memt_cores


# max |device_raw/16 - exact_raw| bound: fp8 quant (~0.13 observed max) +
# bf16 query rounding (2^-8*||m|| ~ 0.09) + f32 accum slack.
DELTA_RAW = np.float32(0.35)


def _exact_topk(qn, img_memory, vals, idx, mnorm):
    """Global exact top-(K+1) per query from device group candidates.

    Device reports top-8 *groups* per block: group g covers rows
    [base+gw*g, base+gw*(g+1)).  Returns (top_vals [B,K+1], top_idx [B,K+1])."""
    # expand groups to rows
    rows_l, vals_l = [], []
    for jb, (base, w, gw) in enumerate(BLOCKS):
        gidx = idx[:, :, jb * 8:(jb + 1) * 8].astype(np.int64)    # [8, B, 8]
        gval = vals[:, :, jb * 8:(jb + 1) * 8]
        core_off = (np.arange(NCORES, dtype=np.int64) * NSHARD)[:, None, None]
        start = core_off + base + gw * gidx                        # [8, B, 8]
        r = start[..., None] + np.arange(gw, dtype=np.int64)       # [8, B, 8, gw]
        v = np.broadcast_to(gval[..., None], r.shape)
        rows_l.append(np.transpose(r, (1, 0, 2, 3)).reshape(B, -1))
        vals_l.append(np.transpose(v, (1, 0, 2, 3)).reshape(B, -1))
    rows = np.concatenate(rows_l, axis=1)                          # [B, ~12.8k]
    rvals = np.concatenate(vals_l, axis=1)
    # upper bound on the true cos of each candidate row (group max + error)
    ub = (rvals + DELTA_RAW) / mnorm[rows]

    # iterative rescore: exact-rescore in ub-descending chunks until the
    # remaining upper bounds cannot beat the current 4th-best exact value
    order_ub = np.argsort(-ub, axis=1)
    ncand = rows.shape[1]
    CH0, CH = 1024, 1024
    top_idx = np.zeros((B, K + 1), np.int64)
    top_val = np.full((B, K + 1), -2.0, np.float32)
    qnorm = np.linalg.norm  # alias

    def _rescore(q, cand_rows):
        rowsf = img_memory[cand_rows]
        rn = rowsf / qnorm(rowsf, axis=1, keepdims=True)
        return (rn @ qn[q].astype(np.float64)).astype(np.float32)

    for q in range(B):
        oq = order_ub[q]
        done = 0
        best_rows = np.empty(0, np.int64)
        best_sims = np.empty(0, np.float32)
        while done < ncand:
            take = CH0 if done == 0 else CH
            chunk = oq[done:done + take]
            done += take
            cr = rows[q, chunk]
            cs = _rescore(q, cr)
            best_rows = np.concatenate([best_rows, cr])
            best_sims = np.concatenate([best_sims, cs])
            o = np.lexsort((best_rows, -best_sims))[:K + 1]
            best_rows, best_sims = best_rows[o], best_sims[o]
            if done < ncand and ub[q, oq[done]] <= best_sims[K] + 1e-7:
                break
        top_idx[q] = best_rows
        top_val[q] = best_sims
    cand = top_idx  # for the fallback path below

    # ---- containment + tie-duplication checks --------------------------------
    v8 = vals[:, :, 7::8]                                          # [8, B, NBLK]
    minn = np.empty((NCORES, NBLK), np.float32)
    maxn = np.empty((NCORES, NBLK), np.float32)
    for c in range(NCORES):
        for jb, (base, w, _) in enumerate(BLOCKS):
            seg = mnorm[c * NSHARD + base: c * NSHARD + base + w]
            minn[c, jb] = seg.min() * (1 - 1e-5)
            maxn[c, jb] = seg.max() * (1 + 1e-5)
    num = v8 + DELTA_RAW
    denom = np.where(num >= 0, minn[:, None, :], maxn[:, None, :])
    ub = num / denom                                               # [8, B, NBLK]
    tau = top_val[:, K]
    viol = set(map(tuple, np.argwhere(ub > (tau[None, :, None] - 1e-6))))

    # FIND_INDEX8 returns the first match per value: exactly-equal f32 group
    # maxes would alias to one index and silently drop a group.
    iview = idx.reshape(NCORES, B, NBLK, 8)
    for c, q, jb in np.argwhere(
            (np.sort(iview, axis=3)[:, :, :, 1:] ==
             np.sort(iview, axis=3)[:, :, :, :-1]).any(axis=3)):
        viol.add((c, q, jb))

    if viol:
        per_q = {}
        for c, q, jb in viol:
            per_q.setdefault(q, set()).add((c, jb))
        for q, blks in per_q.items():
            extra_idx = []
            for c, jb in blks:
                base, w, _ = BLOCKS[jb]
                lo = c * NSHARD + base
                extra_idx.append(np.arange(lo, lo + w, dtype=np.int64))
            extra_idx = np.concatenate(extra_idx + [cand[q]])
            extra_idx = np.unique(extra_idx)
            rowsf = img_memory[extra_idx]
            rn = rowsf / np.linalg.norm(rowsf, axis=1, keepdims=True)
            s = (rn @ qn[q]).astype(np.float32)
            o = np.lexsort((extra_idx, -s))[:K + 1]
            top_idx[q] = extra_idx[o]
            top_val[q] = s[o]
    return top_val, top_idx


def _assemble(i_feats, t_feats, img_memory, txt_memory, top_val, top_idx):
    dt = np.float32
    cand_vals = top_val[:, 1:].astype(dt)                   # [B, K]
    cand_idx = top_idx[:, 1:]
    valid = cand_vals > ID_THRESHOLD

    neg_inf = np.float32(-1e30)
    logits = np.concatenate(
        [np.full((B, 1), SOFT_SCALE, dt),
         np.where(valid, SOFT_SCALE * cand_vals, neg_inf)], axis=1)
    lm = logits.max(axis=1, keepdims=True)
    e = np.exp(logits - lm)
    w = 1.0 - e / e.sum(axis=1, keepdims=True)
    sample_weight = np.where(valid, w[:, 1:], 0.0).astype(dt)

    safe_idx = np.where(valid, cand_idx, 0)
    m = valid[..., None].astype(dt)
    pos_img = img_memory[safe_idx] * m                      # [B, K, D]
    pos_txt = txt_memory[safe_idx] * m

    new_img = np.concatenate([i_feats, pos_img.reshape(B * K, D)], 0).astype(dt)
    new_txt = np.concatenate([t_feats, pos_txt.reshape(B * K, D)], 0).astype(dt)

    qpid = np.arange(B)
    slot_global = np.arange(B * K).reshape(B, K)
    spid = np.where(valid, qpid[:, None], -(slot_global + 1))
    pid = np.concatenate([qpid, spid.reshape(-1)])
    labels = (pid[:, None] == pid[None, :]).astype(dt)

    soft_block = np.zeros((B, B, K), dt)
    soft_block[qpid, qpid, :] = sample_weight
    top = np.concatenate([np.eye(B, dtype=dt), soft_block.reshape(B, B * K)], 1)
    labels[:B, :] = top

    return np.concatenate([new_img, new_txt, labels], axis=0)


def kernel(i_feats, t_feats, img_memory, txt_memory):
    i_feats = np.asarray(i_feats, dtype=np.float32)
    t_feats = np.asarray(t_feats, dtype=np.float32)
    img_memory = np.asarray(img_memory, dtype=np.float32)
    txt_memory = np.asarray(txt_memory, dtype=np.float32)

    qn, qn_bf, qt_host, memt_cores = _prep_inputs(i_feats, img_memory)
    vals, idx, _ = run_device_topk(qt_host, memt_cores, trace=False)

    mnorm = np.sqrt(np.einsum("nd,nd->n", img_memory, img_memory))
    top_val, top_idx = _exact_topk(qn, img_memory, vals, idx, mnorm)
    return _assemble(i_feats, t_feats, img_memory, txt_memory, top_val, top_idx)


# revision 12
# speedup vs baseline: 1.0397x; 1.0397x over previous
"""Sharded kNN retrieval kernel for Trainium2 (8 NeuronCores).

Strategy (v2, fp8 + grouped top-8):
  - Host: l2-normalize queries; queries bf16, img_memory fp8(e4m3, x16 scale);
    per-core transposed layout memt[c] = [128(d-part), 4(d-block), 25000(rows)].
  - Device (SPMD x8), per 2048-col block:
      PE:  sim = qT.T @ memT  (bf16 x fp8 -> PSUM f32)             ~2.1us
      DVE: 16:1 group max (tensor_reduce) straight off PSUM,        ~2.4us
           then MAX8 + FIND_INDEX8 over the 128 group maxes
      DMA: 1.05 MB fp8 per block                                    ~2.6us
    Stages pipeline; each core streams its 12.8 MB shard once.
  - Host: expand group candidates (x16 rows), rank by approx cos, exact f32
    rescore of the top rows; containment + duplicate-index checks with exact
    block-recompute fallback; assemble the reference output exactly.
"""

import numpy as np
import ml_dtypes

import concourse.bass as bass
import concourse.tile as tile
import concourse.mybir as mybir
from concourse import bass_utils

BF16 = ml_dtypes.bfloat16
FP8 = ml_dtypes.float8_e4m3

B = 128
D = 512
N = 200000
NCORES = 8
NSHARD = N // NCORES          # 25000
K = 3
ID_THRESHOLD = 0.15
SOFT_SCALE = 5.0
MSCALE = np.float32(16.0)     # img_memory scaled by 16 before fp8 quantization

WBLK = 2048
_full = NSHARD // WBLK        # 12
_rem = NSHARD - _full * WBLK  # 424
# (base, width, group_width)
BLOCKS = [(j * WBLK, WBLK, 16) for j in range(_full)] + \
         ([(NSHARD - _rem, _rem, 8)] if _rem else [])
NBLK = len(BLOCKS)            # 13

_NC_CACHE = {}


def _build_nc():
    if "nc" in _NC_CACHE:
        return _NC_CACHE["nc"]
    nc = bass.Bass("TRN2", target_bir_lowering=False, debug=False, num_devices=NCORES)
    qt = nc.dram_tensor("qt", [128, 4, 128], mybir.dt.bfloat16, kind="ExternalInput")
    # packed fp8 bytes viewed as u32 (DMA is element-rate-limited; 4x fewer
    # elements): block j occupies u32 cols [base, base+w), chunk-major inside
    memt = nc.dram_tensor("memt", [128, NSHARD], mybir.dt.uint32, kind="ExternalInput")
    vals_out = nc.dram_tensor("vals", [128, NBLK * 8], mybir.dt.float32, kind="ExternalOutput")
    idx_out = nc.dram_tensor("idx", [128, NBLK * 8], mybir.dt.uint16, kind="ExternalOutput")

    NSPLIT = 11 * 8   # result slots for blocks 0..10 (early output)

    with tile.TileContext(nc) as tc:
        with (
            tc.tile_pool(name="qt_pool", bufs=1) as qt_pool,
            tc.tile_pool(name="mem_s", bufs=2) as mem_s,
            tc.tile_pool(name="mem_a", bufs=2) as mem_a,
            tc.tile_pool(name="red_pool", bufs=2) as red_pool,
            tc.tile_pool(name="res_pool", bufs=1) as res_pool,
            tc.tile_pool(name="psum_pool", bufs=2, space="PSUM") as psum_pool,
        ):
            qt_tile = qt_pool.tile([128, 4, 128], mybir.dt.bfloat16)
            nc.scalar.dma_start(qt_tile[:], qt[:])

            vals_a = res_pool.tile([128, NSPLIT], mybir.dt.float32)
            idx_a = res_pool.tile([128, NSPLIT], mybir.dt.uint16)
            vals_b = res_pool.tile([128, NBLK * 8 - NSPLIT], mybir.dt.float32)
            idx_b = res_pool.tile([128, NBLK * 8 - NSPLIT], mybir.dt.uint16)

            for j, (base, w, gw) in enumerate(BLOCKS):
                g = w // gw
                # chunk-major block layout: [chunk][b][sw] fp8 (u32-typed DMA);
                # halves of each full block go to the two HWDGE queues.
                if w == WBLK:
                    tl = mem_s.tile([128, 1024], mybir.dt.uint32, tag="s")
                    nc.sync.dma_start(tl[:], memt[:, base:base + 1024])
                    th = mem_a.tile([128, 1024], mybir.dt.uint32, tag="a")
                    nc.scalar.dma_start(th[:], memt[:, base + 1024:base + 2048])
                    parts = [(tl, 0, 0), (tl, 512, 512), (th, 1024, 0), (th, 1536, 512)]
                else:
                    tl = mem_s.tile([128, w], mybir.dt.uint32, tag="s")
                    nc.sync.dma_start(tl[:], memt[:, base:base + w])
                    parts = [(tl, 0, 0)]
                ps = psum_pool.tile([128, g, gw], mybir.dt.float32, tag="ps")
                for mt, s0, off in parts:
                    sw = min(512, w - s0)
                    for b in range(4):
                        nc.tensor.matmul(
                            ps[:, s0 // gw:(s0 + sw) // gw, :],
                            qt_tile[:, b, :],
                            mt[:, off + b * (sw // 4):off + (b + 1) * (sw // 4)].bitcast(mybir.dt.float8e4),
                            start=(b == 0),
                            stop=(b == 3),
                        )
                red = red_pool.tile([128, g], mybir.dt.float32, tag="red")
                nc.vector.tensor_reduce(red[:], ps[:], axis=mybir.AxisListType.X,
                                        op=mybir.AluOpType.max)
                vt, it, col = (vals_a, idx_a, j * 8) if j * 8 < NSPLIT else \
                              (vals_b, idx_b, j * 8 - NSPLIT)
                nc.vector.max(vt[:, col:col + 8], red[:])
                nc.vector.max_index(it[:, col:col + 8], vt[:, col:col + 8], red[:])
                if j == 10:
                    nc.sync.dma_start(vals_out[:, 0:NSPLIT], vals_a[:])
                    nc.scalar.dma_start(idx_out[:, 0:NSPLIT], idx_a[:])

            nc.sync.dma_start(vals_out[:, NSPLIT:], vals_b[:])
            nc.scalar.dma_start(idx_out[:, NSPLIT:], idx_b[:])
    _split_excess_waits(nc)
    _NC_CACHE["nc"] = nc
    return nc


def _split_excess_waits(nc, keep=1):
    """Walrus's MM instruction struct fits only one embedded sync wait; move
    extra waits emitted by Tile onto standalone NoOps just before the MM."""
    ctr = 0
    for fn in nc.m.functions:
        for blk in fn.blocks:
            newl = []
            for inst in blk.instructions:
                si = inst.sync_info
                if (type(inst).__name__ != "InstNoOp" and si is not None
                        and si.on_wait and len(si.on_wait) > keep):
                    waits = list(si.on_wait)
                    for w in waits[:-keep]:
                        nop = mybir.InstNoOp(name=f"I-waitnop-{ctr}")
                        ctr += 1
                        nop.engine = inst.engine
                        nop.sync_info = mybir.SyncInfo(on_wait=[w], on_update=[])
                        newl.append(nop)
                    inst.sync_info = mybir.SyncInfo(
                        on_wait=waits[-keep:], on_update=list(si.on_update or []))
                newl.append(inst)
            blk.instructions = newl


def run_device_topk(qt_host, memt_cores, trace=False):
    """Run the SPMD device kernel.  Returns (vals [8,128,NBLK*8] f32 raw-dot,
    idx [8,128,NBLK*8] uint32 group indices, BassKernelResults)."""
    nc = _build_nc()
    in_maps = [{"qt": qt_host, "memt": memt_cores[c]} for c in range(NCORES)]
    res = bass_utils.run_bass_kernel_spmd(
        nc, in_maps, core_ids=list(range(NCORES)), trace=trace,
    )
    vals = np.stack([res.results[c]["vals"] for c in range(NCORES)]) / MSCALE
    idx = np.stack([res.results[c]["idx"] for c in range(NCORES)])
    return vals, idx, res


def _prep_inputs(i_feats, img_memory):
    qn = i_feats / np.linalg.norm(i_feats, axis=1, keepdims=True)
    qn = qn.astype(np.float32)
    qn_bf = qn.astype(BF16)
    qt_host = np.ascontiguousarray(qn_bf.reshape(B, 4, 128).transpose(2, 1, 0))

    def _pack(c):
        shard8 = (img_memory[c * NSHARD:(c + 1) * NSHARD] * MSCALE).astype(FP8)
        segs = []
        for base, w, _ in BLOCKS:
            for s0 in range(0, w, 512):         # chunk-major within each block
                sw = min(512, w - s0)
                seg = shard8[base + s0:base + s0 + sw].reshape(sw, 4, 128)
                segs.append(np.ascontiguousarray(seg.transpose(2, 1, 0)).reshape(128, 4 * sw))
        return np.concatenate(segs, axis=1).view(np.uint32)

    from concurrent.futures import ThreadPoolExecutor
    with ThreadPoolExecutor(max_workers=NCORES) as ex:
        memt_cores = list(ex.map(_pack, range(NCORES)))
    return qn, qn_bf, qt_host, memt_cores


# max |device_raw/16 - exact_raw| bound: fp8 quant (~0.13 observed max) +
# bf16 query rounding (2^-8*||m|| ~ 0.09) + f32 accum slack.
DELTA_RAW = np.float32(0.35)


def _exact_topk(qn, img_memory, vals, idx, mnorm):
    """Global exact top-(K+1) per query from device group candidates.

    Device reports top-8 *groups* per block: group g covers rows
    [base+gw*g, base+gw*(g+1)).  Returns (top_vals [B,K+1], top_idx [B,K+1])."""
    # expand groups to rows
    rows_l, vals_l = [], []
    for jb, (base, w, gw) in enumerate(BLOCKS):
        gidx = idx[:, :, jb * 8:(jb + 1) * 8].astype(np.int64)    # [8, B, 8]
        gval = vals[:, :, jb * 8:(jb + 1) * 8]
        core_off = (np.arange(NCORES, dtype=np.int64) * NSHARD)[:, None, None]
        start = core_off + base + gw * gidx                        # [8, B, 8]
        r = start[..., None] + np.arange(gw, dtype=np.int64)       # [8, B, 8, gw]
        v = np.broadcast_to(gval[..., None], r.shape)
        rows_l.append(np.transpose(r, (1, 0, 2, 3)).reshape(B, -1))
        vals_l.append(np.transpose(v, (1, 0, 2, 3)).reshape(B, -1))
    rows = np.concatenate(rows_l, axis=1)                          # [B, ~12.8k]
    rvals = np.concatenate(vals_l, axis=1)
    # upper bound on the true cos of each candidate row (group max + error)
    ub = (rvals + DELTA_RAW) / mnorm[rows]

    # iterative rescore: exact-rescore in ub-descending chunks until the
    # remaining upper bounds cannot beat the current 4th-best exact value
    order_ub = np.argsort(-ub, axis=1)
    ncand = rows.shape[1]
    CH0, CH = 1024, 1024
    top_idx = np.zeros((B, K + 1), np.int64)
    top_val = np.full((B, K + 1), -2.0, np.float32)
    qnorm = np.linalg.norm  # alias

    def _rescore(q, cand_rows):
        rowsf = img_memory[cand_rows]
        rn = rowsf / qnorm(rowsf, axis=1, keepdims=True)
        return (rn @ qn[q].astype(np.float64)).astype(np.float32)

    for q in range(B):
        oq = order_ub[q]
        done = 0
        best_rows = np.empty(0, np.int64)
        best_sims = np.empty(0, np.float32)
        while done < ncand:
            take = CH0 if done == 0 else CH
            chunk = oq[done:done + take]
            done += take
            cr = rows[q, chunk]
            cs = _rescore(q, cr)
            best_rows = np.concatenate([best_rows, cr])
            best_sims = np.concatenate([best_sims, cs])
            o = np.lexsort((best_rows, -best_sims))[:K + 1]
            best_rows, best_sims = best_rows[o], best_sims[o]
            if done < ncand and ub[q, oq[done]] <= best_sims[K] + 1e-7:
                break
        top_idx[q] = best_rows
        top_val[q] = best_sims
    cand = top_idx  # for the fallback path below

    # ---- containment + tie-duplication checks --------------------------------
    v8 = vals[:, :, 7::8]                                          # [8, B, NBLK]
    minn = np.empty((NCORES, NBLK), np.float32)
    maxn = np.empty((NCORES, NBLK), np.float32)
    for c in range(NCORES):
        for jb, (base, w, _) in enumerate(BLOCKS):
            seg = mnorm[c * NSHARD + base: c * NSHARD + base + w]
            minn[c, jb] = seg.min() * (1 - 1e-5)
            maxn[c, jb] = seg.max() * (1 + 1e-5)
    num = v8 + DELTA_RAW
    denom = np.where(num >= 0, minn[:, None, :], maxn[:, None, :])
    ub = num / denom                                               # [8, B, NBLK]
    tau = top_val[:, K]
    viol = set(map(tuple, np.argwhere(ub > (tau[None, :, None] - 1e-6))))

    # FIND_INDEX8 returns the first match per value: exactly-equal f32 group
    # maxes would alias to one index and silently drop a group.
    iview = idx.reshape(NCORES, B, NBLK, 8)
    for c, q, jb in np.argwhere(
            (np.sort(iview, axis=3)[:, :, :, 1:] ==
             np.sort(iview, axis=3)[:, :, :, :-1]).any(axis=3)):
        viol.add((c, q, jb))

    if viol:
        per_q = {}
        for c, q, jb in viol:
            per_q.setdefault(q, set()).add((c, jb))
        for q, blks in per_q.items():
            extra_idx = []
            for c, jb in blks:
                base, w, _ = BLOCKS[jb]
                lo = c * NSHARD + base
                extra_idx.append(np.arange(lo, lo + w, dtype=np.int64))
            extra_idx = np.concatenate(extra_idx + [cand[q]])
            extra_idx = np.unique(extra_idx)
            rowsf = img_memory[extra_idx]
            rn = rowsf / np.linalg.norm(rowsf, axis=1, keepdims=True)
            s = (rn @ qn[q]).astype(np.float32)
            o = np.lexsort((extra_idx, -s))[:K + 1]
            top_idx[q] = extra_idx[o]
            top_val[q] = s[o]
    return top_val, top_idx


def _assemble(i_feats, t_feats, img_memory, txt_memory, top_val, top_idx):
    dt = np.float32
    cand_vals = top_val[:, 1:].astype(dt)                   # [B, K]
    cand_idx = top_idx[:, 1:]
    valid = cand_vals > ID_THRESHOLD

    neg_inf = np.float32(-1e30)
    logits = np.concatenate(
        [np.full((B, 1), SOFT_SCALE, dt),
         np.where(valid, SOFT_SCALE * cand_vals, neg_inf)], axis=1)
    lm = logits.max(axis=1, keepdims=True)
    e = np.exp(logits - lm)
    w = 1.0 - e / e.sum(axis=1, keepdims=True)
    sample_weight = np.where(valid, w[:, 1:], 0.0).astype(dt)

    safe_idx = np.where(valid, cand_idx, 0)
    m = valid[..., None].astype(dt)
    pos_img = img_memory[safe_idx] * m                      # [B, K, D]
    pos_txt = txt_memory[safe_idx] * m

    new_img = np.concatenate([i_feats, pos_img.reshape(B * K, D)], 0).astype(dt)
    new_txt = np.concatenate([t_feats, pos_txt.reshape(B * K, D)], 0).astype(dt)

    qpid = np.arange(B)
    slot_global = np.arange(B * K).reshape(B, K)
    spid = np.where(valid, qpid[:, None], -(slot_global + 1))
    pid = np.concatenate([qpid, spid.reshape(-1)])
    labels = (pid[:, None] == pid[None, :]).astype(dt)

    soft_block = np.zeros((B, B, K), dt)
    soft_block[qpid, qpid, :] = sample_weight
    top = np.concatenate([np.eye(B, dtype=dt), soft_block.reshape(B, B * K)], 1)
    labels[:B, :] = top

    return np.concatenate([new_img, new_txt, labels], axis=0)


def kernel(i_feats, t_feats, img_memory, txt_memory):
    i_feats = np.asarray(i_feats, dtype=np.float32)
    t_feats = np.asarray(t_feats, dtype=np.float32)
    img_memory = np.asarray(img_memory, dtype=np.float32)
    txt_memory = np.asarray(txt_memory, dtype=np.float32)

    qn, qn_bf, qt_host, memt_cores = _prep_inputs(i_feats, img_memory)
    vals, idx, _ = run_device_topk(qt_host, memt_cores, trace=False)

    mnorm = np.sqrt(np.einsum("nd,nd->n", img_memory, img_memory))
    top_val, top_idx = _exact_topk(qn, img_memory, vals, idx, mnorm)
    return _assemble(i_feats, t_feats, img_memory, txt_memory, top_val, top_idx)


# revision 15
# speedup vs baseline: 1.0495x; 1.0094x over previous
"""Sharded kNN retrieval kernel for Trainium2 (8 NeuronCores).

Strategy (v2, fp8 + grouped top-8):
  - Host: l2-normalize queries; queries bf16, img_memory fp8(e4m3, x16 scale);
    per-core transposed layout memt[c] = [128(d-part), 4(d-block), 25000(rows)].
  - Device (SPMD x8), per 2048-col block:
      PE:  sim = qT.T @ memT  (bf16 x fp8 -> PSUM f32)             ~2.1us
      DVE: 16:1 group max (tensor_reduce) straight off PSUM,        ~2.4us
           then MAX8 + FIND_INDEX8 over the 128 group maxes
      DMA: 1.05 MB fp8 per block                                    ~2.6us
    Stages pipeline; each core streams its 12.8 MB shard once.
  - Host: expand group candidates (x16 rows), rank by approx cos, exact f32
    rescore of the top rows; containment + duplicate-index checks with exact
    block-recompute fallback; assemble the reference output exactly.
"""

import numpy as np
import ml_dtypes

import concourse.bass as bass
import concourse.tile as tile
import concourse.mybir as mybir
from concourse import bass_utils

BF16 = ml_dtypes.bfloat16
FP8 = ml_dtypes.float8_e4m3

B = 128
D = 512
N = 200000
NCORES = 8
NSHARD = N // NCORES          # 25000
K = 3
ID_THRESHOLD = 0.15
SOFT_SCALE = 5.0
MSCALE = np.float32(16.0)     # img_memory scaled by 16 before fp8 quantization

WBLK = 2048
_full = NSHARD // WBLK        # 12
_rem = NSHARD - _full * WBLK  # 424
# (base, width, group_width)
BLOCKS = [(j * WBLK, WBLK, 16) for j in range(_full)] + \
         ([(NSHARD - _rem, _rem, 8)] if _rem else [])
NBLK = len(BLOCKS)            # 13

_NC_CACHE = {}


def _build_nc():
    if "nc" in _NC_CACHE:
        return _NC_CACHE["nc"]
    nc = bass.Bass("TRN2", target_bir_lowering=False, debug=False, num_devices=NCORES)
    qt = nc.dram_tensor("qt", [128, 4, 128], mybir.dt.bfloat16, kind="ExternalInput")
    # packed fp8 bytes viewed as u32 (DMA is element-rate-limited; 4x fewer
    # elements): block j occupies u32 cols [base, base+w), chunk-major inside
    memt = nc.dram_tensor("memt", [128, NSHARD], mybir.dt.uint32, kind="ExternalInput")
    # early outputs: blocks 0..10 as [128, 88] f32 tensors; blocks 11-12 are
    # packed (vals | idx-as-f32) into a [128, 32] stage, transposed on DVE to
    # [32, 128] so the final DMA is 32 packets instead of 256.
    NSPLIT = 11 * 8
    vals_out = nc.dram_tensor("vals", [128, NSPLIT], mybir.dt.float32, kind="ExternalOutput")
    idxf_out = nc.dram_tensor("idxf", [128, NSPLIT], mybir.dt.float32, kind="ExternalOutput")
    tail_out = nc.dram_tensor("tail", [32, 128], mybir.dt.float32, kind="ExternalOutput")

    with tile.TileContext(nc) as tc:
        with (
            tc.tile_pool(name="qt_pool", bufs=1) as qt_pool,
            tc.tile_pool(name="mem_s", bufs=5) as mem_s,
            tc.tile_pool(name="mem_a", bufs=5) as mem_a,
            tc.tile_pool(name="red_pool", bufs=4) as red_pool,
            tc.tile_pool(name="res_pool", bufs=1) as res_pool,
            tc.tile_pool(name="psum_pool", bufs=2, space="PSUM") as psum_pool,
        ):
            qt_tile = qt_pool.tile([128, 4, 128], mybir.dt.bfloat16)
            nc.scalar.dma_start(qt_tile[:], qt[:])

            vals_a = res_pool.tile([128, NSPLIT], mybir.dt.float32)
            idxf_a = res_pool.tile([128, NSPLIT], mybir.dt.float32)
            stage = res_pool.tile([128, 32], mybir.dt.float32)
            stageT = res_pool.tile([32, 128], mybir.dt.float32)
            idx8 = res_pool.tile([128, 8], mybir.dt.uint32)

            for j, (base, w, gw) in enumerate(BLOCKS):
                g = w // gw
                # chunk-major block layout: [chunk][b][sw] fp8 (u32-typed DMA);
                # halves of each full block go to the two HWDGE queues (block 0
                # in quarters for a faster pipeline start).
                if j == 0:
                    q = [mem_s.tile([128, 512], mybir.dt.uint32, tag="s", name="q0"),
                         mem_a.tile([128, 512], mybir.dt.uint32, tag="a", name="q1"),
                         mem_s.tile([128, 512], mybir.dt.uint32, tag="s", name="q2"),
                         mem_a.tile([128, 512], mybir.dt.uint32, tag="a", name="q3")]
                    for ci, eng in zip(range(4), (nc.sync, nc.scalar, nc.sync, nc.scalar)):
                        eng.dma_start(q[ci][:], memt[:, base + 512 * ci:base + 512 * (ci + 1)])
                    parts = [(q[ci], 512 * ci, 0) for ci in range(4)]
                elif w == WBLK:
                    tl = mem_s.tile([128, 1024], mybir.dt.uint32, tag="s")
                    nc.sync.dma_start(tl[:], memt[:, base:base + 1024])
                    th = mem_a.tile([128, 1024], mybir.dt.uint32, tag="a")
                    nc.scalar.dma_start(th[:], memt[:, base + 1024:base + 2048])
                    parts = [(tl, 0, 0), (tl, 512, 512), (th, 1024, 0), (th, 1536, 512)]
                else:
                    tl = mem_s.tile([128, w], mybir.dt.uint32, tag="s")
                    nc.sync.dma_start(tl[:], memt[:, base:base + w])
                    parts = [(tl, 0, 0)]
                ps = psum_pool.tile([128, g, gw], mybir.dt.float32, tag="ps")
                for mt, s0, off in parts:
                    sw = min(512, w - s0)
                    for b in range(4):
                        nc.tensor.matmul(
                            ps[:, s0 // gw:(s0 + sw) // gw, :],
                            qt_tile[:, b, :],
                            mt[:, off + b * (sw // 4):off + (b + 1) * (sw // 4)].bitcast(mybir.dt.float8e4),
                            start=(b == 0),
                            stop=(b == 3),
                        )
                red = red_pool.tile([128, g], mybir.dt.float32, tag="red")
                nc.vector.tensor_reduce(red[:], ps[:], axis=mybir.AxisListType.X,
                                        op=mybir.AluOpType.max)
                if j < 11:
                    nc.vector.max(vals_a[:, j * 8:j * 8 + 8], red[:])
                    nc.vector.max_index(idx8[:], vals_a[:, j * 8:j * 8 + 8], red[:])
                    nc.scalar.copy(idxf_a[:, j * 8:j * 8 + 8], idx8[:])
                else:
                    col = (j - 11) * 8
                    nc.vector.max(stage[:, col:col + 8], red[:])
                    nc.vector.max_index(idx8[:], stage[:, col:col + 8], red[:])
                    nc.scalar.copy(stage[:, 16 + col:24 + col], idx8[:])
                if j == 10:
                    nc.sync.dma_start(vals_out[:], vals_a[:])
                    nc.scalar.dma_start(idxf_out[:], idxf_a[:])

            for s in range(4):
                nc.vector.transpose(stageT[0:32, 32 * s:32 * s + 32],
                                    stage[32 * s:32 * s + 32, 0:32])
            nc.sync.dma_start(tail_out[:], stageT[:])
    _split_excess_waits(nc)
    _NC_CACHE["nc"] = nc
    return nc


def _split_excess_waits(nc, keep=1):
    """Walrus's MM instruction struct fits only one embedded sync wait; move
    extra waits emitted by Tile onto standalone NoOps just before the MM."""
    ctr = 0
    for fn in nc.m.functions:
        for blk in fn.blocks:
            newl = []
            for inst in blk.instructions:
                si = inst.sync_info
                if (type(inst).__name__ != "InstNoOp" and si is not None
                        and si.on_wait and len(si.on_wait) > keep):
                    waits = list(si.on_wait)
                    for w in waits[:-keep]:
                        nop = mybir.InstNoOp(name=f"I-waitnop-{ctr}")
                        ctr += 1
                        nop.engine = inst.engine
                        nop.sync_info = mybir.SyncInfo(on_wait=[w], on_update=[])
                        newl.append(nop)
                    inst.sync_info = mybir.SyncInfo(
                        on_wait=waits[-keep:], on_update=list(si.on_update or []))
                newl.append(inst)
            blk.instructions = newl


def run_device_topk(qt_host, memt_cores, trace=False):
    """Run the SPMD device kernel.  Returns (vals [8,128,NBLK*8] f32 raw-dot,
    idx [8,128,NBLK*8] uint32 group indices, BassKernelResults)."""
    nc = _build_nc()
    in_maps = [{"qt": qt_host, "memt": memt_cores[c]} for c in range(NCORES)]
    res = bass_utils.run_bass_kernel_spmd(
        nc, in_maps, core_ids=list(range(NCORES)), trace=trace,
    )
    vals = np.empty((NCORES, 128, NBLK * 8), np.float32)
    idx = np.empty((NCORES, 128, NBLK * 8), np.uint32)
    for c in range(NCORES):
        r = res.results[c]
        stage = r["tail"].T                       # [128, 32]
        vals[c] = np.concatenate([r["vals"], stage[:, 0:16]], axis=1) / MSCALE
        idx[c] = np.rint(np.concatenate(
            [r["idxf"], stage[:, 16:32]], axis=1)).astype(np.uint32)
    return vals, idx, res


def _prep_inputs(i_feats, img_memory):
    qn = i_feats / np.linalg.norm(i_feats, axis=1, keepdims=True)
    qn = qn.astype(np.float32)
    qn_bf = qn.astype(BF16)
    qt_host = np.ascontiguousarray(qn_bf.reshape(B, 4, 128).transpose(2, 1, 0))

    def _pack(c):
        shard8 = (img_memory[c * NSHARD:(c + 1) * NSHARD] * MSCALE).astype(FP8)
        segs = []
        for base, w, _ in BLOCKS:
            for s0 in range(0, w, 512):         # chunk-major within each block
                sw = min(512, w - s0)
                seg = shard8[base + s0:base + s0 + sw].reshape(sw, 4, 128)
                segs.append(np.ascontiguousarray(seg.transpose(2, 1, 0)).reshape(128, 4 * sw))
        return np.concatenate(segs, axis=1).view(np.uint32)

    from concurrent.futures import ThreadPoolExecutor
    with ThreadPoolExecutor(max_workers=NCORES) as ex:
        memt_cores = list(ex.map(_pack, range(NCORES)))
    return qn, qn_bf, qt_host, memt_cores


# max |device_raw/16 - exact_raw| bound: fp8 quant (~0.13 observed max) +
# bf16 query rounding (2^-8*||m|| ~ 0.09) + f32 accum slack.
DELTA_RAW = np.float32(0.35)


def _exact_topk(qn, img_memory, vals, idx, mnorm):
    """Global exact top-(K+1) per query from device group candidates.

    Device reports top-8 *groups* per block: group g covers rows
    [base+gw*g, base+gw*(g+1)).  Returns (top_vals [B,K+1], top_idx [B,K+1])."""
    # expand groups to rows
    rows_l, vals_l = [], []
    for jb, (base, w, gw) in enumerate(BLOCKS):
        gidx = idx[:, :, jb * 8:(jb + 1) * 8].astype(np.int64)    # [8, B, 8]
        gval = vals[:, :, jb * 8:(jb + 1) * 8]
        core_off = (np.arange(NCORES, dtype=np.int64) * NSHARD)[:, None, None]
        start = core_off + base + gw * gidx                        # [8, B, 8]
        r = start[..., None] + np.arange(gw, dtype=np.int64)       # [8, B, 8, gw]
        v = np.broadcast_to(gval[..., None], r.shape)
        rows_l.append(np.transpose(r, (1, 0, 2, 3)).reshape(B, -1))
        vals_l.append(np.transpose(v, (1, 0, 2, 3)).reshape(B, -1))
    rows = np.concatenate(rows_l, axis=1)                          # [B, ~12.8k]
    rvals = np.concatenate(vals_l, axis=1)
    # upper bound on the true cos of each candidate row (group max + error)
    ub = (rvals + DELTA_RAW) / mnorm[rows]

    # iterative rescore: exact-rescore in ub-descending chunks until the
    # remaining upper bounds cannot beat the current 4th-best exact value
    order_ub = np.argsort(-ub, axis=1)
    ncand = rows.shape[1]
    CH0, CH = 1024, 1024
    top_idx = np.zeros((B, K + 1), np.int64)
    top_val = np.full((B, K + 1), -2.0, np.float32)
    qnorm = np.linalg.norm  # alias

    def _rescore(q, cand_rows):
        rowsf = img_memory[cand_rows]
        rn = rowsf / qnorm(rowsf, axis=1, keepdims=True)
        return (rn @ qn[q].astype(np.float64)).astype(np.float32)

    for q in range(B):
        oq = order_ub[q]
        done = 0
        best_rows = np.empty(0, np.int64)
        best_sims = np.empty(0, np.float32)
        while done < ncand:
            take = CH0 if done == 0 else CH
            chunk = oq[done:done + take]
            done += take
            cr = rows[q, chunk]
            cs = _rescore(q, cr)
            best_rows = np.concatenate([best_rows, cr])
            best_sims = np.concatenate([best_sims, cs])
            o = np.lexsort((best_rows, -best_sims))[:K + 1]
            best_rows, best_sims = best_rows[o], best_sims[o]
            if done < ncand and ub[q, oq[done]] <= best_sims[K] + 1e-7:
                break
        top_idx[q] = best_rows
        top_val[q] = best_sims
    cand = top_idx  # for the fallback path below

    # ---- containment + tie-duplication checks --------------------------------
    v8 = vals[:, :, 7::8]                                          # [8, B, NBLK]
    minn = np.empty((NCORES, NBLK), np.float32)
    maxn = np.empty((NCORES, NBLK), np.float32)
    for c in range(NCORES):
        for jb, (base, w, _) in enumerate(BLOCKS):
            seg = mnorm[c * NSHARD + base: c * NSHARD + base + w]
            minn[c, jb] = seg.min() * (1 - 1e-5)
            maxn[c, jb] = seg.max() * (1 + 1e-5)
    num = v8 + DELTA_RAW
    denom = np.where(num >= 0, minn[:, None, :], maxn[:, None, :])
    ub = num / denom                                               # [8, B, NBLK]
    tau = top_val[:, K]
    viol = set(map(tuple, np.argwhere(ub > (tau[None, :, None] - 1e-6))))

    # FIND_INDEX8 returns the first match per value: exactly-equal f32 group
    # maxes would alias to one index and silently drop a group.
    iview = idx.reshape(NCORES, B, NBLK, 8)
    for c, q, jb in np.argwhere(
            (np.sort(iview, axis=3)[:, :, :, 1:] ==
             np.sort(iview, axis=3)[:, :, :, :-1]).any(axis=3)):
        viol.add((c, q, jb))

    if viol:
        per_q = {}
        for c, q, jb in viol:
            per_q.setdefault(q, set()).add((c, jb))
        for q, blks in per_q.items():
            extra_idx = []
            for c, jb in blks:
                base, w, _ = BLOCKS[jb]
                lo = c * NSHARD + base
                extra_idx.append(np.arange(lo, lo + w, dtype=np.int64))
            extra_idx = np.concatenate(extra_idx + [cand[q]])
            extra_idx = np.unique(extra_idx)
            rowsf = img_memory[extra_idx]
            rn = rowsf / np.linalg.norm(rowsf, axis=1, keepdims=True)
            s = (rn @ qn[q]).astype(np.float32)
            o = np.lexsort((extra_idx, -s))[:K + 1]
            top_idx[q] = extra_idx[o]
            top_val[q] = s[o]
    return top_val, top_idx


def _assemble(i_feats, t_feats, img_memory, txt_memory, top_val, top_idx):
    dt = np.float32
    cand_vals = top_val[:, 1:].astype(dt)                   # [B, K]
    cand_idx = top_idx[:, 1:]
    valid = cand_vals > ID_THRESHOLD

    neg_inf = np.float32(-1e30)
    logits = np.concatenate(
        [np.full((B, 1), SOFT_SCALE, dt),
         np.where(valid, SOFT_SCALE * cand_vals, neg_inf)], axis=1)
    lm = logits.max(axis=1, keepdims=True)
    e = np.exp(logits - lm)
    w = 1.0 - e / e.sum(axis=1, keepdims=True)
    sample_weight = np.where(valid, w[:, 1:], 0.0).astype(dt)

    safe_idx = np.where(valid, cand_idx, 0)
    m = valid[..., None].astype(dt)
    pos_img = img_memory[safe_idx] * m                      # [B, K, D]
    pos_txt = txt_memory[safe_idx] * m

    new_img = np.concatenate([i_feats, pos_img.reshape(B * K, D)], 0).astype(dt)
    new_txt = np.concatenate([t_feats, pos_txt.reshape(B * K, D)], 0).astype(dt)

    qpid = np.arange(B)
    slot_global = np.arange(B * K).reshape(B, K)
    spid = np.where(valid, qpid[:, None], -(slot_global + 1))
    pid = np.concatenate([qpid, spid.reshape(-1)])
    labels = (pid[:, None] == pid[None, :]).astype(dt)

    soft_block = np.zeros((B, B, K), dt)
    soft_block[qpid, qpid, :] = sample_weight
    top = np.concatenate([np.eye(B, dtype=dt), soft_block.reshape(B, B * K)], 1)
    labels[:B, :] = top

    return np.concatenate([new_img, new_txt, labels], axis=0)


def kernel(i_feats, t_feats, img_memory, txt_memory):
    i_feats = np.asarray(i_feats, dtype=np.float32)
    t_feats = np.asarray(t_feats, dtype=np.float32)
    img_memory = np.asarray(img_memory, dtype=np.float32)
    txt_memory = np.asarray(txt_memory, dtype=np.float32)

    qn, qn_bf, qt_host, memt_cores = _prep_inputs(i_feats, img_memory)
    vals, idx, _ = run_device_topk(qt_host, memt_cores, trace=False)

    mnorm = np.sqrt(np.einsum("nd,nd->n", img_memory, img_memory))
    top_val, top_idx = _exact_topk(qn, img_memory, vals, idx, mnorm)
    return _assemble(i_feats, t_feats, img_memory, txt_memory, top_val, top_idx)
